# revision 41
# baseline (speedup 1.0000x reference)
"""Causal self-attention (B=8, T=1024, C=1024, H=16) on 8 trn2 NeuronCores.

Data-parallel over batch: each core computes one batch element's full
attention layer; no collectives. All matmuls in bf16 (2 rows/cycle).

Host pre-transposes inputs so every contraction dim lands on partitions:
  xT    [C, T]        x[b].T
  wqk   [C, 8, 256]   w_attn[:2C].T, k/q packed per head-pair
  wvT   [C, C]        w_attn[2C:].T
  wpT   [C, C]        w_proj.T
  bb    [128, C]      b_proj broadcast along partitions
  triT  [128, 128]    multiplicative causal mask (bf16) in [j, i] orientation

Schedule (tuned against perfetto traces over ~10 iterations):
  primer   throwaway matmuls on memset data keep the PE HAM clock-gate
           warm from ~7.5us (no DMA dependency) while x/wqk stream in
  DMA      only the critical 5MB (x, wvT, wqk[0:2]) up front, spread over
           all three DMA queues (~100GB/s each), wqk/wpT host-swizzled so
           every transfer is contiguous; wpT/bias issued later
  kqproj(0) first (needs only x + wqk[0], the earliest DMAs); km/qm kept
           as four half-tiles so QK's first blocks depend on only two of
           the four chains
  V-proj   v[t, vj] = xT.T @ wvT as v_aug tiles [t, pair, v|1|v|1],
           emitted inside pair 0 so its matmuls overlap pair-0's exps
  per pair m: the two heads' QK^T matmuls are interleaved (even head on PE
    rows 0:64, odd on 64:128 -> concurrent row groups); exp(s/8) straight
    from PSUM (3-deep wide att ring; all narrow PSUM shares the ya ring);
    causality is a POST-exp bf16 triangular multiply (gpsimd for h0, DVE
    for h1), off the ACT critical path; AV per head: yT_aug[65, i] +=
    v_aug.T @ pT (row 64 = softmax denom via the ones column); the next
    pair's first QK blocks are slipped into AV's tail so the next exp
    group starts promptly; denominator recip + partition-broadcast via a
    DRAM reshape hop (pairs 0-6, fully overlapped) or, for the last pair,
    an on-chip path: ones-weight fp32r matmul broadcasts the denom row,
    then DVE reciprocal + multiply, so the output projection is not
    stalled behind DMA round trips
  out      out[t, :] = YT.T @ wpT + b in wide att-ring PSUM tiles with a
           3-tile lookahead on the m=0..6 partial sums, m=7 last, so
           ~9us of matmuls cover pair 7's normalize latency
"""
import sys
from contextlib import ExitStack

sys.path.insert(0, "/opt/trn_rl_repo")
import numpy as np
import ml_dtypes

from concourse import bacc, mybir
from concourse import tile
from concourse.bass_utils import run_bass_kernel_spmd

B, T, C = 8, 1024, 1024
H = 16
D = C // H  # 64
NCORES = 8
NPAIR = H // 2  # 8
NTB = T // 128  # 8
NCB = C // 128  # 8
F32 = mybir.dt.float32
F32R = mybir.dt.float32r
BF16 = mybir.dt.bfloat16
AF = mybir.ActivationFunctionType
SCALE = 1.0 / 8.0  # 1/sqrt(D)


def r(ap):
    return ap.bitcast(F32R)


def self_norm(nc, smp, yt, yr, hp, rb):
    """yt rows for head hp = yr rows 0:64 scaled by the recip-denom rb.

    hp=1 lands on partitions 64:128, which no compute engine can shift
    to, so it bounces through a small SBUF->SBUF DMA.
    """
    if hp == 0:
        for ch in range(2):
            nc.vector.tensor_mul(
                yt[0:64, ch * 512:(ch + 1) * 512],
                yr[0:64, ch * 512:(ch + 1) * 512], rb[ch])
    else:
        ytmp = smp.tile([64, T], BF16, name="ytmp", tag="ytmp", bufs=2)
        for ch in range(2):
            nc.vector.tensor_mul(
                ytmp[:, ch * 512:(ch + 1) * 512],
                yr[0:64, ch * 512:(ch + 1) * 512], rb[ch])
        nc.sync.dma_start(out=yt[64:128, :], in_=ytmp[:])


def build():
    nc = bacc.Bacc(target_bir_lowering=False)
    xT = nc.dram_tensor("xT", [C, T], BF16, kind="ExternalInput")
    wqk = nc.dram_tensor("wqk", [NPAIR, 128, 2, NCB, 128], BF16,
                         kind="ExternalInput")
    wvT = nc.dram_tensor("wvT", [C, C], BF16, kind="ExternalInput")
    wpT = nc.dram_tensor("wpT", [128, NCB, C], BF16, kind="ExternalInput")
    bb = nc.dram_tensor("bb", [128, C], F32, kind="ExternalInput")
    triT = nc.dram_tensor("triT", [128, 128], BF16, kind="ExternalInput")
    out = nc.dram_tensor("out", [T, C], F32, kind="ExternalOutput")

    with tile.TileContext(nc) as tc, ExitStack() as top:
        const = top.enter_context(tc.tile_pool(name="const", bufs=1))
        ytp = top.enter_context(tc.tile_pool(name="yt", bufs=1))
        smp = top.enter_context(tc.tile_pool(name="sm", bufs=1))
        psb = top.enter_context(tc.tile_pool(name="psb", bufs=2, space="PSUM"))
        dramp = top.enter_context(tc.tile_pool(name="dram", bufs=1, space="DRAM"))

        # ---- DMA-independent primer data (memset) ----
        pmw = const.tile([128, 128], BF16, name="pmw")
        nc.vector.memset(pmw[:], 0.03)
        pmx = const.tile([128, 512], BF16, name="pmx")
        nc.vector.memset(pmx[:], 0.03)
        # ones row at partition 64 (must match the denom row's base
        # partition for the broadcast matmul)
        ones_st = const.tile([65, 64], F32, name="ones_st")
        nc.vector.memset(ones_st[:], 1.0)
        ones64 = const.tile([65, 64], F32, name="ones64")
        nc.vector.tensor_copy(r(ones64[:]), ones_st[:])

        trit = const.tile([128, 128], BF16, name="trit")
        bbt = const.tile([128, C], F32, name="bbt")
        wpt = const.tile([128, NCB, C], BF16, name="wpt")

        yts = []
        with ExitStack() as mid:
            xp = mid.enter_context(tc.tile_pool(name="xp", bufs=1))
            vtp = mid.enter_context(tc.tile_pool(name="vt", bufs=1))
            wq = mid.enter_context(tc.tile_pool(name="wq", bufs=1))
            kqp = mid.enter_context(tc.tile_pool(name="kq", bufs=1))
            ptp = mid.enter_context(tc.tile_pool(name="pt", bufs=1))
            wvp = mid.enter_context(tc.tile_pool(name="wv", bufs=1))

            wqms = {}

            def load_wqm(mm_, eng):
                # host-swizzled contiguous layout; split k / q halves into
                # separate DMAs so the k-pass starts as soon as k lands
                wqm_ = wq.tile([128, 2, NCB, 128], BF16, name=f"wqm{mm_}",
                               tag="wqm", bufs=2)
                eng.dma_start(out=wqm_[:, 0], in_=wqk[mm_, :, 0])
                eng.dma_start(out=wqm_[:, 1], in_=wqk[mm_, :, 1])
                wqms[mm_] = wqm_

            # ---- DMA issue order: ONLY the early-needed 5MB up front
            # (x + wvT on scalar/sync, wqk[0,1] on gpsimd); bias and
            # out-proj weights are issued later in program order so they
            # don't steal HBM bandwidth from the critical initial load ----
            # k-half of pair-0 weights first (kqproj(0) needs it), then all
            # of x round-robin (everything needs x), then the rest
            wqm0 = wq.tile([128, 2, NCB, 128], BF16, name="wqm0",
                           tag="wqm", bufs=2)
            nc.gpsimd.dma_start(out=wqm0[:, 0], in_=wqk[0, :, 0])
            wqms[0] = wqm0
            q3 = [nc.scalar, nc.sync, nc.gpsimd]
            xts = []
            for cb in range(NCB):
                xt = xp.tile([128, T], BF16, name=f"x{cb}", tag="x", bufs=NCB)
                q3[cb % 3].dma_start(out=xt[:],
                                     in_=xT[cb * 128:(cb + 1) * 128, :])
                xts.append(xt)
            nc.gpsimd.dma_start(out=wqm0[:, 1], in_=wqk[0, :, 1])
            wvt = wvp.tile([128, NCB, C], BF16, name="wvt")
            for cb in range(NCB):
                q3[cb % 3].dma_start(out=wvt[:, cb, :],
                                     in_=wvT[cb * 128:(cb + 1) * 128, :])
            load_wqm(1, nc.gpsimd)
            nc.sync.dma_start(out=trit[:], in_=triT[:])

            # ---- primer: keep the PE busy (and the HAM un-throttled)
            # until the x DMAs land. Results are never read. ----
            prim = psb.tile([128, 512], F32, name="prim", tag="ya", bufs=2)
            for _ in range(22):
                nc.tensor.matmul(prim[:, 0:512], pmw[:], pmx[:],
                                 start=True, stop=True)

            kqs = {}

            def kqproj(mm_):
                # km/qm as four independent half-tiles: QK's first blocks
                # only depend on the th=0 halves, so the exp stream starts
                # as soon as those two chains (not all four) finish
                wqm_ = wqms[mm_]
                halves = []
                for kq in range(2):  # k pass, q pass
                    for th in range(2):
                        h = kqp.tile([128, 512], BF16,
                                     name=f"kq{mm_}_{kq}{th}",
                                     tag=f"kq{kq}{th}", bufs=2)
                        ps = psb.tile([128, 512], F32, name="kqps",
                                      tag="ya", bufs=2)
                        for cb in range(NCB):
                            nc.tensor.matmul(
                                ps[:],
                                wqm_[:, kq, cb, :],
                                xts[cb][:, th * 512:(th + 1) * 512],
                                start=(cb == 0), stop=(cb == NCB - 1))
                        nc.vector.tensor_copy(h[:], ps[:])
                        halves.append(h)
                kqs[mm_] = halves  # [k0, k1, q0, q1]

            # pair-0 K/Q projection first: it only needs x + wqk[0], the
            # earliest DMAs, so the PE has real work while wvT lands
            kqproj(0)

            # ---- V projection (invoked INSIDE pair 0, between its QK and
            # AV, so pair-0's ~12us of exps overlap the V matmuls) ----
            vts = []

            def vproj():
                # out-proj weights + bias: issue now, HBM is quiet
                nc.gpsimd.dma_start(out=wpt[:], in_=wpT[:])
                nc.scalar.dma_start(out=bbt[:], in_=bb[:])
                for tb in range(NTB):
                    vt = vtp.tile([128, NPAIR, 130], BF16, name=f"v{tb}",
                                  tag="v", bufs=NTB)
                    for half in range(2):
                        # ya-ring, not att-ring: pair-0's QK tiles hold the
                        # att ring until their exps drain, and AV (the next
                        # ya user) only starts after V-proj anyway
                        ps = psb.tile([128, 512], F32, name="vps",
                                      tag="ya", bufs=2)
                        for cb in range(NCB):
                            nc.tensor.matmul(
                                ps[:, 0:512],
                                xts[cb][:, tb * 128:(tb + 1) * 128],
                                wvt[:, cb, half * 512:(half + 1) * 512],
                                start=(cb == 0), stop=(cb == NCB - 1))
                        psv = ps[:, 0:512].rearrange("p (pr f) -> p pr f",
                                                     f=128)
                        nc.vector.tensor_copy(
                            vt[:, half * 4:(half + 1) * 4, 0:64],
                            psv[:, :, 0:64])
                        nc.vector.tensor_copy(
                            vt[:, half * 4:(half + 1) * 4, 65:129],
                            psv[:, :, 64:128])
                    vt2 = vt.rearrange("p pr (two f) -> p (pr two) f", f=65)
                    nc.vector.memset(vt2[:, :, 64:65], 1.0)
                    vts.append(vt)

            # ---- attention, per head pair, software-pipelined: pair m's
            # body runs QK(m) (feeding the exp stream), kqproj(m+1), then
            # AV(m-1) -- whose exps all completed during the PREVIOUS
            # body, so nothing in a body waits on this pair's exps and
            # the ACT engine never idles between exp groups ----
            def tri_mul(hp, ap):
                # causality: zero the upper triangle of the diagonal
                # block AFTER exp (bf16 multiply, off the ACT critical
                # path; denom comes from AV so it sees the zeroed
                # values). Split across gpsimd and DVE so neither stream
                # paces the exps; h1 (whose tri gates the AV tail) gets
                # the lower-latency DVE.
                eng = nc.gpsimd if hp == 0 else nc.vector
                eng.tensor_mul(ap, ap, trit[:])

            def qk_phase(m, jbs, pts=None):
                k0, k1, q0, q1 = kqs[m]
                # QK^T interleaved across the two heads: even head in PE
                # rows 0:64, odd head in rows 64:128 -> concurrent.
                # ACTIVATE has a ~258ns fixed cost and ACT is the pacing
                # engine, so exps are merged where tiles allow: jb4 packs
                # both heads in one [128,1024] tile (1 exp), jb5-7 pack
                # into one [128,768] tile per head (1 exp per head).
                # pts[hp][jb] = (tile, column offset of jb's block).
                if pts is None:
                    pts = {0: {}, 1: {}}
                for jb in jbs:
                    w = T - jb * 128
                    if jb == 4:
                        att = psb.tile([128, 1024], F32, name="att",
                                       tag="att", bufs=3)
                        pt = ptp.tile([128, 1024], BF16, name="pt4",
                                      tag="pt4", bufs=3)
                        for hp in range(2):
                            p0 = hp * 64
                            nc.tensor.matmul(
                                att[:, hp * 512:hp * 512 + 512],
                                k1[p0:p0 + 64, 0:128],
                                q1[p0:p0 + 64, 0:512],
                                start=True, stop=True)
                            pts[hp][4] = (pt, hp * 512)
                        nc.scalar.activation(pt[:], att[:], AF.Exp,
                                             scale=SCALE)
                        for hp in range(2):
                            tri_mul(hp, pt[:, hp * 512:hp * 512 + 128])
                    elif jb == 5:
                        # bank-aligned: a matmul output must not cross the
                        # 512-col PSUM bank boundary (cols 384:512 unused)
                        offs = {5: 0, 6: 512, 7: 768}
                        for hp in range(2):
                            p0 = hp * 64
                            att = psb.tile([128, 1024], F32, name="att",
                                           tag="att", bufs=3)
                            pt = ptp.tile([128, 896], BF16, name="pt567",
                                          tag="pt567", bufs=3)
                            for j2 in (5, 6, 7):
                                w2 = T - j2 * 128
                                nc.tensor.matmul(
                                    att[:, offs[j2]:offs[j2] + w2],
                                    k1[p0:p0 + 64,
                                       j2 * 128 - 512:j2 * 128 - 384],
                                    q1[p0:p0 + 64,
                                       j2 * 128 - 512:j2 * 128 - 512 + w2],
                                    start=True, stop=True)
                                pts[hp][j2] = (pt, offs[j2])
                            nc.scalar.activation(pt[:, 0:896],
                                                 att[:, 0:896], AF.Exp,
                                                 scale=SCALE)
                            for j2 in (5, 6, 7):
                                tri_mul(hp, pt[:, offs[j2]:offs[j2] + 128])
                    elif jb in (6, 7):
                        continue  # handled with jb 5
                    else:
                        for hp in range(2):
                            p0 = hp * 64
                            pt = ptp.tile([128, w], BF16,
                                          name=f"pt{jb}_{hp}",
                                          tag=f"pt{jb}", bufs=3)
                            pts[hp][jb] = (pt, 0)
                            att = psb.tile([128, 1024], F32, name="att",
                                           tag="att", bufs=3)
                            kh = k0 if jb < 4 else k1
                            kc = jb * 128 - (512 if jb >= 4 else 0)
                            for ch in range(2):
                                i0 = max(jb * 128, ch * 512)
                                cw = (ch + 1) * 512 - i0
                                if cw <= 0:
                                    continue
                                qh = q0 if ch == 0 else q1
                                nc.tensor.matmul(
                                    att[:, i0:i0 + cw],
                                    kh[p0:p0 + 64, kc:kc + 128],
                                    qh[p0:p0 + 64,
                                       i0 - ch * 512:i0 - ch * 512 + cw],
                                    start=True, stop=True)
                            nc.scalar.activation(
                                pt[:, 0:w], att[:, jb * 128:T], AF.Exp,
                                scale=SCALE)
                            tri_mul(hp, pt[:, 0:128])
                return pts

            def av_and_norm(m, pts, mid_hook=None):
                yt = ytp.tile([128, T], BF16, name=f"yt{m}", tag="yt",
                              bufs=NPAIR)
                yts.append(yt)
                last_pair = m == NPAIR - 1
                yrs = []
                for hp in range(2):  # AV per head
                    voff = 65 * hp
                    ya = [psb.tile([128, 512], F32, name="ya",
                                   tag="ya", bufs=2) for _ in range(2)]
                    for jb in range(NTB):
                        if hp == 1 and jb == 5 and mid_hook is not None:
                            # slip the next pair's first QK blocks in here
                            # so its exp group starts with no boundary gap
                            mid_hook()
                        for ch in range(2):
                            if jb * 128 >= (ch + 1) * 512:
                                continue
                            i0 = max(jb * 128, ch * 512)
                            cw = (ch + 1) * 512 - i0
                            first = (jb == 0)
                            last = (jb == NTB - 1) or \
                                (ch == 0 and jb == 3)
                            ptile, poff = pts[hp][jb]
                            pc = poff + i0 - jb * 128
                            nc.tensor.matmul(
                                ya[ch][0:65,
                                       i0 - ch * 512:i0 - ch * 512 + cw],
                                vts[jb][:, m, voff:voff + 65],
                                ptile[:, pc:pc + cw],
                                start=first, stop=last)
                    # evict psum fast: yr rows 0:64 = raw y, row 64 = denom
                    yr = smp.tile([65, T], F32, name="yr", tag="yr",
                                  bufs=4)
                    for ch in range(2):
                        # last pair: evict on gpsimd (frees the DVE for the
                        # recip+mul chain) through an fp32r view so the
                        # denom row is legal input for the fp32r broadcast
                        # matmul (rounding is harmless for the y values)
                        dst = yr[0:65, ch * 512:(ch + 1) * 512]
                        if last_pair:
                            dst = r(dst)
                        nc.vector.tensor_copy(dst, ya[ch][0:65, 0:512])
                    yrs.append(yr)
                    if not last_pair:
                        # denom -> DRAM -> [128, 8] -> recip -> DRAM -> bcast
                        # (4 DMA hops, but fully overlapped mid-run)
                        dd0 = dramp.tile([1, T], F32, name="dd0", tag="dd0",
                                         bufs=2)
                        nc.sync.dma_start(out=dd0[:], in_=yr[64:65, :])
                        dtr = smp.tile([128, 8], F32, name="dtr", tag="dtr",
                                       bufs=2)
                        nc.sync.dma_start(
                            out=dtr[:],
                            in_=dd0[0, :].rearrange("(p q) -> p q", q=8))
                        rtr = smp.tile([128, 8], F32, name="rtr", tag="rtr",
                                       bufs=2)
                        nc.vector.reciprocal(rtr[:], dtr[:])
                        dd = dramp.tile([1, T], F32, name="dd", tag="dd",
                                        bufs=2)
                        nc.sync.dma_start(
                            out=dd[0, :].rearrange("(p q) -> p q", q=8),
                            in_=rtr[:])
                        bcm = smp.tile([64, T], F32, name="bc", tag="bc",
                                       bufs=3)
                        nc.sync.dma_start(
                            out=bcm[:], in_=dd[0, :].partition_broadcast(64))
                        self_norm(nc, smp, yt, yr, hp,
                                  [bcm[:, ch * 512:(ch + 1) * 512]
                                   for ch in range(2)])
                if last_pair:
                    # low-latency on-chip denom path so the output
                    # projection is not stalled behind DMA round trips:
                    # ones-weight matmuls broadcast both heads' denom rows
                    # back-to-back (PE stays busy), then DVE reciprocal
                    bcds = []
                    for hp in range(2):
                        pair_b = []
                        for ch in range(2):
                            bcd = psb.tile([128, 512], F32, name="bcd",
                                           tag="ya", bufs=2)
                            nc.tensor.matmul(
                                bcd[0:64, 0:512], r(ones64[64:65, :]),
                                r(yrs[hp][64:65, ch * 512:(ch + 1) * 512]),
                                start=True, stop=True)
                            pair_b.append(bcd)
                        bcds.append(pair_b)
                    for hp in range(2):
                        rb = []
                        for ch in range(2):
                            rbc = smp.tile([64, 512], F32, name="rbc",
                                           tag="rbc", bufs=4)
                            nc.vector.reciprocal(
                                rbc[:], bcds[hp][ch][0:64, 0:512])
                            rb.append(rbc[:])
                        self_norm(nc, smp, yt, yrs[hp], hp, rb)

            ops = {}

            def op_partial(tb):
                op = psb.tile([128, 1024], F32, name="op", tag="att",
                              bufs=3)
                ops[tb] = op
                for mo in range(NPAIR - 1):
                    for half in range(2):
                        nc.tensor.matmul(
                            op[:, half * 512:(half + 1) * 512],
                            yts[mo][:, tb * 128:(tb + 1) * 128],
                            wpt[:, mo, half * 512:(half + 1) * 512],
                            start=(mo == 0), stop=False)

            # high priority: pair-0's QK must precede the (wvT-gated)
            # V-proj matmuls in the in-order PE stream, or the first exp
            # group starts ~18us late
            with tc.high_priority():
                pts = qk_phase(0, range(NTB))
            vproj()  # V matmuls run while pair-0 exps drain
            for m in range(NPAIR):
                if m + 1 < NPAIR:
                    kqproj(m + 1)  # PE filler while exps drain
                    nxt = {0: {}, 1: {}}
                    hook = (lambda mm=m + 1, pp=nxt:
                            qk_phase(mm, range(0, 2), pp))
                else:
                    nxt, hook = None, None
                av_and_norm(m, pts, hook)
                if m + 1 < NPAIR:
                    qk_phase(m + 1, range(2, NTB), nxt)
                pts = nxt
                if m + 2 < NPAIR:
                    load_wqm(m + 2, nc.gpsimd)

        # ---- output projection ----
        # both halves live in one wide PSUM tile from the (now free) att
        # ring; run with a 3-tile lookahead on the pair-0..6 partial sums
        # so ~9us of matmuls cover pair 7's normalize latency, and the
        # m=7 contribution lands last
        with tc.tile_pool(name="os", bufs=2) as osp:
            for tb in range(3):
                op_partial(tb)
            for tb in range(NTB):
                op = ops.pop(tb)
                m = NPAIR - 1
                for half in range(2):
                    nc.tensor.matmul(
                        op[:, half * 512:(half + 1) * 512],
                        yts[m][:, tb * 128:(tb + 1) * 128],
                        wpt[:, m, half * 512:(half + 1) * 512],
                        start=False, stop=True)
                if tb + 3 < NTB:
                    op_partial(tb + 3)
                ost = osp.tile([128, C], F32, name="ost", tag="ost", bufs=2)
                for half in range(2):
                    nc.vector.tensor_add(
                        ost[:, half * 512:(half + 1) * 512],
                        op[:, half * 512:(half + 1) * 512],
                        bbt[:, half * 512:(half + 1) * 512])
                eng = nc.sync if tb % 2 == 0 else nc.scalar
                eng.dma_start(
                    out=out[tb * 128:(tb + 1) * 128, :], in_=ost[:])

    nc.compile()
    return nc


_NC = None


def _get_nc():
    global _NC
    if _NC is None:
        _NC = build()
    return _NC


def prep_inputs(x, w_attn, w_proj, b_proj):
    x = np.asarray(x, dtype=np.float32)
    w_attn = np.asarray(w_attn, dtype=np.float32)
    w_proj = np.asarray(w_proj, dtype=np.float32)
    b_proj = np.asarray(b_proj, dtype=np.float32)
    BF = ml_dtypes.bfloat16
    # wqk: [NPAIR, 128(p), 2(k|q), NCB, 128] so each per-pair DMA is
    # contiguous (2KB per partition)
    ki = w_attn[0:C].T.reshape(NCB, 128, NPAIR, 128).transpose(2, 1, 0, 3)
    qi = w_attn[C:2 * C].T.reshape(NCB, 128, NPAIR, 128).transpose(2, 1, 0, 3)
    wqkv = np.ascontiguousarray(
        np.stack([ki, qi], axis=2)).astype(BF)  # [m, p, 2, cb, 128]
    wvTv = np.ascontiguousarray(w_attn[2 * C:3 * C].T).astype(BF)
    # wpT: [128(p), NCB, C] so the prefetch DMA is one contiguous
    # 16KB-per-partition transfer
    wpTv = np.ascontiguousarray(
        w_proj.T.reshape(NCB, 128, C).transpose(1, 0, 2)).astype(BF)
    bbv = np.broadcast_to(b_proj, (128, C)).copy()
    ii = np.arange(128)
    tri = np.where(ii[None, :] >= ii[:, None], 1.0, 0.0).astype(BF)
    shared = {"wqk": wqkv, "wvT": wvTv, "wpT": wpTv, "bb": bbv, "triT": tri}
    in_maps = []
    for b in range(B):
        im = dict(shared)
        im["xT"] = np.ascontiguousarray(x[b].T).astype(ml_dtypes.bfloat16)
        in_maps.append(im)
    return in_maps


def kernel(x, w_attn, w_proj, b_proj):
    nc = _get_nc()
    in_maps = prep_inputs(x, w_attn, w_proj, b_proj)
    res = run_bass_kernel_spmd(nc, in_maps, core_ids=list(range(NCORES)))
    return np.stack([res.results[b]["out"] for b in range(B)]).astype(np.float32)
